# revision 17
# baseline (speedup 1.0000x reference)
"""Trainium2 Bass kernel for nn_CrossNetMix (low-rank mixture-of-experts CrossNet).

Reference math per cross layer i (E=4 experts, rank R=64, C=512, B=16384):
    gate = xi @ gate_w.T                         # [B, E]
    v    = tanh(einsum('bc,ecr->ber', xi, Vs[i]))
    cv   = tanh(einsum('ber,erq->beq', v, Cs[i]))
    ucv  = einsum('ber,ecr->bec', cv, Us[i])
    xi   = einsum('bec,be->bc', x0[:,None,:]*(ucv + biases[i]), gate) + xi

Kernel restructuring:
  *  xi_{l+1} = x0 ** (1 + sum_{j<=l} S_j),  S_l = sum_e gate_e*(U_e @ cv_e)
     (+ bias_l * sum_e gate_e when biases nonzero), so the mm3 outputs
     accumulate in PSUM across all 3 layers and a single fused
     scalar_tensor_tensor produces each layer's xi.
  *  Everything runs transposed (features on partitions, batch on the
     moving/free dim) so the three matmuls chain with no transposes.
     Host pre-transposes x (free) and the small weights.
  *  gate_w columns are replicated R times inside mm1's weights, so the
     gate comes out of mm1 pre-broadcast as [E*R, b], ready for the
     per-expert scale of cv.
  *  All matmul operands use float32r (~17-bit mantissa, 4x faster than
     fp32 on the PE at moving-dim 512).

Sharding: data-parallel over batch across 8 NeuronCores (2048 rows each);
weights replicated. No collectives.
"""

import numpy as np

import concourse.bass as bass
import concourse.tile as tile
from concourse import mybir
from concourse.alu_op_type import AluOpType
from concourse.bass_utils import run_bass_kernel_spmd

# Problem shape (hardcoded per contract).
B, C, R, E, L = 16384, 512, 64, 4, 3
NCORES = 8
BC = B // NCORES          # batch per core
CHUNK = 512               # moving-dim tile (fp32/f32r matmul max)
NCH = BC // CHUNK         # batch chunks per core
CT = C // 128             # feature tiles (partition dim)
ER = E * R                # 256

F32 = mybir.dt.float32
F32R = mybir.dt.float32r
TANH = mybir.ActivationFunctionType.Tanh


def _split_multiwait(nc, max_keep=1):
    """walrus in this toolchain rejects instructions carrying >1 sem wait
    (the Tile final drain gets several). Hoist extras onto 1-wait NOPs."""
    for f in nc.m.functions:
        for bb in f.blocks:
            out = []
            for inst in bb.instructions:
                si = inst.sync_info
                if si is not None and si.on_wait and len(si.on_wait) > max_keep:
                    waits = list(si.on_wait)
                    extra, keep = waits[:-max_keep], waits[-max_keep:]
                    for w in extra:
                        nop = mybir.InstNoOp(
                            name=nc.get_next_instruction_name(),
                            ins=[], outs=[], engine=inst.engine)
                        nop.sync_info = mybir.SyncInfo(on_wait=[w], on_update=[])
                        out.append(nop)
                    si.on_wait = keep
                out.append(inst)
            bb.instructions = out


def build_kernel(with_bias=False):
    nc = bass.Bass(trn_type="TRN2")
    xT = nc.dram_tensor("xT", [C, BC], F32R, kind="ExternalInput")
    w1 = nc.dram_tensor("w1", [L, 128, 4 * CT * 128], F32R, kind="ExternalInput")
    w2 = nc.dram_tensor("w2", [L, 128, 2 * 2 * 128], F32R, kind="ExternalInput")
    w3 = nc.dram_tensor("w3", [L, 128, 2 * CT * 128], F32R, kind="ExternalInput")
    if with_bias:
        # bias[l] as a single-partition row [1, C] (lhsT for rank-1 matmuls)
        bia = nc.dram_tensor("biasT", [L, 1, C], F32R, kind="ExternalInput")
    outT = nc.dram_tensor("outT", [C, BC], F32, kind="ExternalOutput")

    with tile.TileContext(nc) as tc:
        with (
            tc.tile_pool(name="wp", bufs=1) as wp,
            tc.tile_pool(name="x0p", bufs=1) as x0p,
            tc.tile_pool(name="xip", bufs=2) as xip,
            tc.tile_pool(name="vp", bufs=2) as vp,
            tc.tile_pool(name="op", bufs=2) as op_pool,
            tc.tile_pool(name="hps", bufs=2, space=bass.MemorySpace.PSUM) as hps,
            tc.tile_pool(name="cvps", bufs=2, space=bass.MemorySpace.PSUM) as cvps,
            tc.tile_pool(name="sps", bufs=1, space=bass.MemorySpace.PSUM) as sps,
        ):
            # DMA order matters: compute can start once layer-0 weights and
            # chunk-0 x0 land, so interleave weights-by-layer with x0-by-chunk.
            x0t = {}
            W1t, W2t, W3t, Bt = [], [], [], []

            def load_weights(l, eng1=None, eng23=None):
                e1 = eng1 or nc.sync
                e23 = eng23 or nc.sync
                t1 = wp.tile([128, 4 * CT * 128], F32R, tag=f"w1_{l}",
                             name=f"w1_{l}")
                if eng1 is not None:
                    # per-k-tile DMAs so the first matmul only waits ~256KB
                    for k in range(CT):
                        e1.dma_start(
                            t1[:, k * 512:(k + 1) * 512],
                            w1[l][:, k * 512:(k + 1) * 512])
                else:
                    e1.dma_start(t1[:], w1[l])
                W1t.append(t1)
                t2 = wp.tile([128, 2 * 2 * 128], F32R, tag=f"w2_{l}",
                             name=f"w2_{l}")
                e23.dma_start(t2[:], w2[l])
                W2t.append(t2)
                t3 = wp.tile([128, 2 * CT * 128], F32R, tag=f"w3_{l}",
                             name=f"w3_{l}")
                e23.dma_start(t3[:], w3[l])
                W3t.append(t3)
                if with_bias:
                    tb = wp.tile([1, C], F32R, tag=f"bias_{l}", name=f"b_{l}")
                    nc.sync.dma_start(tb[:], bia[l])
                    Bt.append(tb)

            def load_x0(ch, eng=None):
                e = eng or nc.sync
                for ct in range(CT):
                    t = x0p.tile([128, CHUNK], F32R, tag=f"x0_{ct}_{ch}",
                                 name=f"x0_{ct}_{ch}")
                    e.dma_start(
                        t[:], xT[ct * 128:(ct + 1) * 128,
                                  ch * CHUNK:(ch + 1) * CHUNK])
                    x0t[(ct, ch)] = t

            # Head-critical tiles go out on SWDGE queues triggered from
            # otherwise-idle engines (first HWDGE byte is ~9us out; SWDGE
            # starts ~1us after trigger). Everything else streams on HWDGE.
            load_weights(0, eng1=nc.scalar, eng23=nc.scalar)
            load_x0(0, eng=nc.gpsimd)
            load_weights(1)
            load_x0(1)
            load_weights(2)
            load_x0(2)
            load_x0(3)

            # PE clock pre-warm: HAM un-throttles (1.2 -> 2.4 GHz) only after
            # ~3.4us of sustained PE activity. Burn ~5us of bf16 dummy
            # matmuls during the DMA head so real matmuls start warm.
            warm_sb = vp.tile([128, 128], mybir.dt.bfloat16, tag="warm",
                              name="warm_sb")
            nc.gpsimd.memset(warm_sb[:], 0.0)
            warm_ps = hps.tile([128, 128], F32, tag="h", name="warm_ps")
            for _ in range(36):
                nc.tensor.matmul(warm_ps[:], warm_sb[:], warm_sb[:],
                                 start=True, stop=True)

            def w1_ap(l, k, m):
                return W1t[l][:, (k * CT + m) * 128:(k * CT + m + 1) * 128]

            def w2_ap(l, k, m):
                return W2t[l][:, (k * 2 + m) * 128:(k * 2 + m + 1) * 128]

            def w3_ap(l, k, m):
                return W3t[l][:, (k * CT + m) * 128:(k * CT + m + 1) * 128]

            for ch in range(NCH):
                sacc = [sps.tile([128, CHUNK], F32, tag=f"sacc{m}", name=f"sacc{m}")
                        for m in range(CT)]
                xi = [x0t[(ct, ch)][:] for ct in range(CT)]
                x0sl = list(xi)
                for l in range(L):
                    # ---- mm1, V part (out rows 0..255) -> tanh -> vT ----
                    # k-rotation: m-group m starts on k=m, so after a layer
                    # boundary each group begins on the earliest-ready xiT
                    # tile instead of all groups serializing on STT(k=0..3).
                    h_ps = [hps.tile([128, CHUNK], F32, tag="h", name="hps")
                            for _ in range(2)]
                    for m in (0, 1):
                        korder = [(m + j) % CT for j in range(CT)]
                        for j, k in enumerate(korder):
                            nc.tensor.matmul(h_ps[m][:], w1_ap(l, k, m), xi[k],
                                             start=(j == 0), stop=(j == CT - 1))
                    v_sb = []
                    for m in (0, 1):
                        v = vp.tile([128, CHUNK], F32R, tag=f"v{m}")
                        nc.scalar.activation(v[:], h_ps[m][:], TANH)
                        v_sb.append(v)

                    # ---- mm1, replicated-gate part (rows 256..383) ----
                    gb_ps = [None, None]
                    gb_ps[0] = hps.tile([128, CHUNK], F32, tag="h", name="gbps")
                    for j, k in enumerate([(2 + j) % CT for j in range(CT)]):
                        nc.tensor.matmul(gb_ps[0][:], w1_ap(l, k, 2), xi[k],
                                         start=(j == 0), stop=(j == CT - 1))

                    # ---- mm2 (block-diag C) ----
                    cv_ps = [cvps.tile([128, CHUNK], F32, tag="cv", name="cvps")
                             for _ in range(2)]
                    for m in (0, 1):
                        ko = (0, 1) if m == 0 else (1, 0)
                        for j, k in enumerate(ko):
                            nc.tensor.matmul(cv_ps[m][:], w2_ap(l, k, m),
                                             v_sb[k][:],
                                             start=(j == 0), stop=(j == 1))

                    # gate rows 384..511 after mm2: keeps the PE busy during
                    # the tanh(cv) -> scale serial chain
                    gb_ps[1] = hps.tile([128, CHUNK], F32, tag="h", name="gbps")
                    for j, k in enumerate([(3 + j) % CT for j in range(CT)]):
                        nc.tensor.matmul(gb_ps[1][:], w1_ap(l, k, 3), xi[k],
                                         start=(j == 0), stop=(j == CT - 1))

                    # ---- tanh -> cv; scale by gate ----
                    scaled = []
                    for m in (0, 1):
                        cv = vp.tile([128, CHUNK], F32R, tag=f"cv{m}")
                        nc.scalar.activation(cv[:], cv_ps[m][:], TANH)
                        s = vp.tile([128, CHUNK], F32R, tag=f"s{m}")
                        nc.vector.tensor_tensor(s[:], cv[:], gb_ps[m][:],
                                                op=AluOpType.mult)
                        scaled.append(s)

                    if with_bias:
                        # G[b] = sum_e gate[e, b] from the replicated-gate psum
                        # rows {0, 64} of each gb tile; rank-1 bias update
                        # sacc += biasT[l] (x) G via a K=1 matmul.
                        g01 = vp.tile([1, CHUNK], F32R, tag="g01")
                        g23 = vp.tile([1, CHUNK], F32R, tag="g23")
                        g = vp.tile([1, CHUNK], F32R, tag="g")
                        nc.vector.tensor_tensor(
                            g01[:], gb_ps[0][0:1, :], gb_ps[0][64:65, :],
                            op=AluOpType.add)
                        nc.vector.tensor_tensor(
                            g23[:], gb_ps[1][0:1, :], gb_ps[1][64:65, :],
                            op=AluOpType.add)
                        nc.vector.tensor_tensor(
                            g[:], g01[:], g23[:], op=AluOpType.add)

                    # ---- mm3 accumulates S across layers in PSUM; the xi
                    # update STT for each c-tile is emitted right after its
                    # m-group so the next layer's mm1 unblocks ASAP ----
                    nxt = []
                    for m in range(CT):
                        ko = (0, 1) if m % 2 == 0 else (1, 0)
                        for j, k in enumerate(ko):
                            nc.tensor.matmul(
                                sacc[m][:], w3_ap(l, k, m), scaled[k][:],
                                start=(l == 0 and j == 0),
                                stop=(l == L - 1 and j == 1 and not with_bias),
                                skip_group_check=True)
                        if with_bias:
                            nc.tensor.matmul(
                                sacc[m][:],
                                Bt[l][0:1, m * 128:(m + 1) * 128],
                                g[:],
                                start=False, stop=(l == L - 1),
                                skip_group_check=True)
                        # xi_{l+1} = (sacc + 1) * x0 (fused STT)
                        if l < L - 1:
                            t = xip.tile([128, CHUNK], F32R, tag=f"xi{m}",
                                         name=f"xi{m}")
                            nc.vector.scalar_tensor_tensor(
                                t[:], sacc[m][:], 1.0, x0sl[m],
                                op0=AluOpType.add, op1=AluOpType.mult)
                            nxt.append(t[:])
                        else:
                            o = op_pool.tile([128, CHUNK], F32, tag=f"o{m}",
                                             name=f"o{m}")
                            nc.vector.scalar_tensor_tensor(
                                o[:], sacc[m][:], 1.0, x0sl[m],
                                op0=AluOpType.add, op1=AluOpType.mult)
                            nc.sync.dma_start(
                                outT[m * 128:(m + 1) * 128,
                                     ch * CHUNK:(ch + 1) * CHUNK], o[:])
                    if l < L - 1:
                        xi = nxt
    _split_multiwait(nc)
    return nc


def _pack_lhsT(w_full, kt, mt):
    """[K, M] weight -> [128, kt*mt*128] SBUF lhsT tile layout."""
    K, M = w_full.shape
    assert K == kt * 128 and M == mt * 128
    return np.ascontiguousarray(
        w_full.reshape(kt, 128, mt, 128).transpose(1, 0, 2, 3)
        .reshape(128, kt * mt * 128).astype(np.float32))


def _pack_weights(Us, Vs, Cs, gate_w):
    w1 = np.zeros((L, 128, 4 * CT * 128), np.float32)
    w2 = np.zeros((L, 128, 2 * 2 * 128), np.float32)
    w3 = np.zeros((L, 128, 2 * CT * 128), np.float32)
    gate_rep = np.repeat(gate_w, R, axis=0).T            # [C, ER]
    for l in range(L):
        w1_full = np.concatenate(
            [Vs[l].transpose(1, 0, 2).reshape(C, ER), gate_rep], axis=1)
        w1[l] = _pack_lhsT(w1_full, CT, CT)
        w2_full = np.zeros((ER, ER), np.float32)
        for e in range(E):
            w2_full[e * R:(e + 1) * R, e * R:(e + 1) * R] = Cs[l, e]
        w2[l] = _pack_lhsT(w2_full, 2, 2)
        w3[l] = _pack_lhsT(Us[l].transpose(0, 2, 1).reshape(ER, C), 2, CT)
    return w1, w2, w3


_CACHED = {}


def _get_nc(with_bias):
    if with_bias not in _CACHED:
        _CACHED[with_bias] = build_kernel(with_bias)
    return _CACHED[with_bias]


def run(inputs, trace=False, trace_kwargs=None):
    x = np.asarray(inputs["x"], np.float32)
    Us = np.asarray(inputs["Us"], np.float32)
    Vs = np.asarray(inputs["Vs"], np.float32)
    Cs = np.asarray(inputs["Cs"], np.float32)
    gate_w = np.asarray(inputs["gate_w"], np.float32)
    biases = np.asarray(inputs["biases"], np.float32)
    with_bias = bool(np.any(biases))

    w1, w2, w3 = _pack_weights(Us, Vs, Cs, gate_w)
    nc = _get_nc(with_bias)

    in_maps = []
    for h in range(NCORES):
        m = {
            "xT": np.ascontiguousarray(x[h * BC:(h + 1) * BC].T),
            "w1": w1, "w2": w2, "w3": w3,
        }
        if with_bias:
            m["biasT"] = np.ascontiguousarray(biases.reshape(L, 1, C))
        in_maps.append(m)

    kw = {}
    if trace:
        kw.update(trace=True, **(trace_kwargs or {}))
    res = run_bass_kernel_spmd(nc, in_maps, core_ids=list(range(NCORES)), **kw)
    out = np.concatenate(
        [res.results[h]["outT"].T for h in range(NCORES)], axis=0)
    return np.ascontiguousarray(out, dtype=np.float32), res


def kernel(**inputs) -> np.ndarray:
    out, _ = run(inputs, trace=False)
    return out


# revision 18
# speedup vs baseline: 1.0498x; 1.0498x over previous
"""Trainium2 Bass kernel for nn_CrossNetMix (low-rank mixture-of-experts CrossNet).

Reference math per cross layer i (E=4 experts, rank R=64, C=512, B=16384):
    gate = xi @ gate_w.T                         # [B, E]
    v    = tanh(einsum('bc,ecr->ber', xi, Vs[i]))
    cv   = tanh(einsum('ber,erq->beq', v, Cs[i]))
    ucv  = einsum('ber,ecr->bec', cv, Us[i])
    xi   = einsum('bec,be->bc', x0[:,None,:]*(ucv + biases[i]), gate) + xi

Kernel restructuring:
  *  xi_{l+1} = x0 ** (1 + sum_{j<=l} S_j),  S_l = sum_e gate_e*(U_e @ cv_e)
     (+ bias_l * sum_e gate_e when biases nonzero), so the mm3 outputs
     accumulate in PSUM across all 3 layers and a single fused
     scalar_tensor_tensor produces each layer's xi.
  *  Everything runs transposed (features on partitions, batch on the
     moving/free dim) so the three matmuls chain with no transposes.
     Host pre-transposes x (free) and the small weights.
  *  gate_w columns are replicated R times inside mm1's weights, so the
     gate comes out of mm1 pre-broadcast as [E*R, b], ready for the
     per-expert scale of cv.
  *  All matmul operands use float32r (~17-bit mantissa, 4x faster than
     fp32 on the PE at moving-dim 512).

Sharding: data-parallel over batch across 8 NeuronCores (2048 rows each);
weights replicated. No collectives.
"""

import numpy as np

import concourse.bass as bass
import concourse.tile as tile
from concourse import mybir
from concourse.alu_op_type import AluOpType
from concourse.bass_utils import run_bass_kernel_spmd

# Problem shape (hardcoded per contract).
B, C, R, E, L = 16384, 512, 64, 4, 3
NCORES = 8
BC = B // NCORES          # batch per core
CHUNK = 512               # moving-dim tile (fp32/f32r matmul max)
NCH = BC // CHUNK         # batch chunks per core
CT = C // 128             # feature tiles (partition dim)
ER = E * R                # 256

F32 = mybir.dt.float32
F32R = mybir.dt.float32r
TANH = mybir.ActivationFunctionType.Tanh


def _split_multiwait(nc, max_keep=1):
    """walrus in this toolchain rejects instructions carrying >1 sem wait
    (the Tile final drain gets several). Hoist extras onto 1-wait NOPs."""
    for f in nc.m.functions:
        for bb in f.blocks:
            out = []
            for inst in bb.instructions:
                si = inst.sync_info
                if si is not None and si.on_wait and len(si.on_wait) > max_keep:
                    waits = list(si.on_wait)
                    extra, keep = waits[:-max_keep], waits[-max_keep:]
                    for w in extra:
                        nop = mybir.InstNoOp(
                            name=nc.get_next_instruction_name(),
                            ins=[], outs=[], engine=inst.engine)
                        nop.sync_info = mybir.SyncInfo(on_wait=[w], on_update=[])
                        out.append(nop)
                    si.on_wait = keep
                out.append(inst)
            bb.instructions = out


def build_kernel(with_bias=False):
    nc = bass.Bass(trn_type="TRN2")
    xT = nc.dram_tensor("xT", [C, BC], F32R, kind="ExternalInput")
    w1 = nc.dram_tensor("w1", [L, 128, 4 * CT * 128], F32R, kind="ExternalInput")
    w2 = nc.dram_tensor("w2", [L, 128, 2 * 2 * 128], F32R, kind="ExternalInput")
    w3 = nc.dram_tensor("w3", [L, 128, 2 * CT * 128], F32R, kind="ExternalInput")
    if with_bias:
        # bias[l] as a single-partition row [1, C] (lhsT for rank-1 matmuls)
        bia = nc.dram_tensor("biasT", [L, 1, C], F32R, kind="ExternalInput")
    outT = nc.dram_tensor("outT", [C, BC], F32, kind="ExternalOutput")

    with tile.TileContext(nc) as tc:
        with (
            tc.tile_pool(name="wp", bufs=1) as wp,
            tc.tile_pool(name="x0p", bufs=1) as x0p,
            tc.tile_pool(name="xip", bufs=2) as xip,
            tc.tile_pool(name="vp", bufs=2) as vp,
            tc.tile_pool(name="op", bufs=2) as op_pool,
            tc.tile_pool(name="hps", bufs=2, space=bass.MemorySpace.PSUM) as hps,
            tc.tile_pool(name="cvps", bufs=2, space=bass.MemorySpace.PSUM) as cvps,
            tc.tile_pool(name="sps", bufs=1, space=bass.MemorySpace.PSUM) as sps,
        ):
            # DMA order matters: compute can start once layer-0 weights and
            # chunk-0 x0 land, so interleave weights-by-layer with x0-by-chunk.
            x0t = {}
            W1t, W2t, W3t, Bt = [], [], [], []

            def load_weights(l, eng1=None, eng23=None):
                e1 = eng1 or nc.sync
                e23 = eng23 or nc.sync
                t1 = wp.tile([128, 4 * CT * 128], F32R, tag=f"w1_{l}",
                             name=f"w1_{l}")
                if eng1 is not None:
                    # per-k-tile DMAs so the first matmul only waits ~256KB
                    for k in range(CT):
                        e1.dma_start(
                            t1[:, k * 512:(k + 1) * 512],
                            w1[l][:, k * 512:(k + 1) * 512])
                else:
                    e1.dma_start(t1[:], w1[l])
                W1t.append(t1)
                t2 = wp.tile([128, 2 * 2 * 128], F32R, tag=f"w2_{l}",
                             name=f"w2_{l}")
                e23.dma_start(t2[:], w2[l])
                W2t.append(t2)
                t3 = wp.tile([128, 2 * CT * 128], F32R, tag=f"w3_{l}",
                             name=f"w3_{l}")
                e23.dma_start(t3[:], w3[l])
                W3t.append(t3)
                if with_bias:
                    tb = wp.tile([1, C], F32R, tag=f"bias_{l}", name=f"b_{l}")
                    nc.sync.dma_start(tb[:], bia[l])
                    Bt.append(tb)

            def load_x0(ch, eng=None):
                e = eng or nc.sync
                for ct in range(CT):
                    t = x0p.tile([128, CHUNK], F32R, tag=f"x0_{ct}_{ch}",
                                 name=f"x0_{ct}_{ch}")
                    e.dma_start(
                        t[:], xT[ct * 128:(ct + 1) * 128,
                                  ch * CHUNK:(ch + 1) * CHUNK])
                    x0t[(ct, ch)] = t

            load_weights(0, eng1=nc.sync)  # split per-k so k0 lands first
            load_x0(0)
            load_weights(1)
            load_x0(1)
            load_weights(2)
            load_x0(2)
            load_x0(3)

            # PE clock pre-warm: HAM un-throttles (1.2 -> 2.4 GHz) only after
            # ~3.4us of sustained PE activity. Bridge the DMA head (~8us)
            # with bf16 dummy matmuls so real matmuls start at full clock.
            warm_sb = vp.tile([128, 128], mybir.dt.bfloat16, tag="warm",
                              name="warm_sb")
            nc.vector.memset(warm_sb[:], 0.0)
            warm_ps = hps.tile([128, 128], F32, tag="h", name="warm_ps")
            for _ in range(75):
                nc.tensor.matmul(warm_ps[:], warm_sb[:], warm_sb[:],
                                 start=True, stop=True)

            def w1_ap(l, k, m):
                return W1t[l][:, (k * CT + m) * 128:(k * CT + m + 1) * 128]

            def w2_ap(l, k, m):
                return W2t[l][:, (k * 2 + m) * 128:(k * 2 + m + 1) * 128]

            def w3_ap(l, k, m):
                return W3t[l][:, (k * CT + m) * 128:(k * CT + m + 1) * 128]

            for ch in range(NCH):
                sacc = [sps.tile([128, CHUNK], F32, tag=f"sacc{m}", name=f"sacc{m}")
                        for m in range(CT)]
                xi = [x0t[(ct, ch)][:] for ct in range(CT)]
                x0sl = list(xi)
                for l in range(L):
                    # ---- mm1, V part (out rows 0..255) -> tanh -> vT ----
                    # k-rotation: m-group m starts on k=m, so after a layer
                    # boundary each group begins on the earliest-ready xiT
                    # tile instead of all groups serializing on STT(k=0..3).
                    h_ps = [hps.tile([128, CHUNK], F32, tag="h", name="hps")
                            for _ in range(2)]
                    for m in (0, 1):
                        korder = [(m + j) % CT for j in range(CT)]
                        for j, k in enumerate(korder):
                            nc.tensor.matmul(h_ps[m][:], w1_ap(l, k, m), xi[k],
                                             start=(j == 0), stop=(j == CT - 1))
                    v_sb = []
                    for m in (0, 1):
                        v = vp.tile([128, CHUNK], F32R, tag=f"v{m}")
                        nc.scalar.activation(v[:], h_ps[m][:], TANH)
                        v_sb.append(v)

                    # ---- mm1, replicated-gate part (rows 256..383) ----
                    gb_ps = [None, None]
                    gb_ps[0] = hps.tile([128, CHUNK], F32, tag="h", name="gbps")
                    for j, k in enumerate([(2 + j) % CT for j in range(CT)]):
                        nc.tensor.matmul(gb_ps[0][:], w1_ap(l, k, 2), xi[k],
                                         start=(j == 0), stop=(j == CT - 1))

                    # ---- mm2 (block-diag C) ----
                    cv_ps = [cvps.tile([128, CHUNK], F32, tag="cv", name="cvps")
                             for _ in range(2)]
                    for m in (0, 1):
                        ko = (0, 1) if m == 0 else (1, 0)
                        for j, k in enumerate(ko):
                            nc.tensor.matmul(cv_ps[m][:], w2_ap(l, k, m),
                                             v_sb[k][:],
                                             start=(j == 0), stop=(j == 1))

                    # gate rows 384..511 after mm2: keeps the PE busy during
                    # the tanh(cv) -> scale serial chain
                    gb_ps[1] = hps.tile([128, CHUNK], F32, tag="h", name="gbps")
                    for j, k in enumerate([(3 + j) % CT for j in range(CT)]):
                        nc.tensor.matmul(gb_ps[1][:], w1_ap(l, k, 3), xi[k],
                                         start=(j == 0), stop=(j == CT - 1))

                    # ---- tanh -> cv; scale by gate ----
                    scaled = []
                    for m in (0, 1):
                        cv = vp.tile([128, CHUNK], F32R, tag=f"cv{m}")
                        nc.scalar.activation(cv[:], cv_ps[m][:], TANH)
                        s = vp.tile([128, CHUNK], F32R, tag=f"s{m}")
                        nc.vector.tensor_tensor(s[:], cv[:], gb_ps[m][:],
                                                op=AluOpType.mult)
                        scaled.append(s)

                    if with_bias:
                        # G[b] = sum_e gate[e, b] from the replicated-gate psum
                        # rows {0, 64} of each gb tile; rank-1 bias update
                        # sacc += biasT[l] (x) G via a K=1 matmul.
                        g01 = vp.tile([1, CHUNK], F32R, tag="g01")
                        g23 = vp.tile([1, CHUNK], F32R, tag="g23")
                        g = vp.tile([1, CHUNK], F32R, tag="g")
                        nc.vector.tensor_tensor(
                            g01[:], gb_ps[0][0:1, :], gb_ps[0][64:65, :],
                            op=AluOpType.add)
                        nc.vector.tensor_tensor(
                            g23[:], gb_ps[1][0:1, :], gb_ps[1][64:65, :],
                            op=AluOpType.add)
                        nc.vector.tensor_tensor(
                            g[:], g01[:], g23[:], op=AluOpType.add)

                    # ---- mm3 accumulates S across layers in PSUM; the xi
                    # update STT for each c-tile is emitted right after its
                    # m-group so the next layer's mm1 unblocks ASAP ----
                    nxt = []
                    for m in range(CT):
                        ko = (0, 1) if m % 2 == 0 else (1, 0)
                        for j, k in enumerate(ko):
                            nc.tensor.matmul(
                                sacc[m][:], w3_ap(l, k, m), scaled[k][:],
                                start=(l == 0 and j == 0),
                                stop=(l == L - 1 and j == 1 and not with_bias),
                                skip_group_check=True)
                        if with_bias:
                            nc.tensor.matmul(
                                sacc[m][:],
                                Bt[l][0:1, m * 128:(m + 1) * 128],
                                g[:],
                                start=False, stop=(l == L - 1),
                                skip_group_check=True)
                        # xi_{l+1} = (sacc + 1) * x0 (fused STT)
                        if l < L - 1:
                            t = xip.tile([128, CHUNK], F32R, tag=f"xi{m}",
                                         name=f"xi{m}")
                            nc.vector.scalar_tensor_tensor(
                                t[:], sacc[m][:], 1.0, x0sl[m],
                                op0=AluOpType.add, op1=AluOpType.mult)
                            nxt.append(t[:])
                        else:
                            o = op_pool.tile([128, CHUNK], F32, tag=f"o{m}",
                                             name=f"o{m}")
                            nc.vector.scalar_tensor_tensor(
                                o[:], sacc[m][:], 1.0, x0sl[m],
                                op0=AluOpType.add, op1=AluOpType.mult)
                            nc.sync.dma_start(
                                outT[m * 128:(m + 1) * 128,
                                     ch * CHUNK:(ch + 1) * CHUNK], o[:])
                    if l < L - 1:
                        xi = nxt
    _split_multiwait(nc)
    return nc


def _pack_lhsT(w_full, kt, mt):
    """[K, M] weight -> [128, kt*mt*128] SBUF lhsT tile layout."""
    K, M = w_full.shape
    assert K == kt * 128 and M == mt * 128
    return np.ascontiguousarray(
        w_full.reshape(kt, 128, mt, 128).transpose(1, 0, 2, 3)
        .reshape(128, kt * mt * 128).astype(np.float32))


def _pack_weights(Us, Vs, Cs, gate_w):
    w1 = np.zeros((L, 128, 4 * CT * 128), np.float32)
    w2 = np.zeros((L, 128, 2 * 2 * 128), np.float32)
    w3 = np.zeros((L, 128, 2 * CT * 128), np.float32)
    gate_rep = np.repeat(gate_w, R, axis=0).T            # [C, ER]
    for l in range(L):
        w1_full = np.concatenate(
            [Vs[l].transpose(1, 0, 2).reshape(C, ER), gate_rep], axis=1)
        w1[l] = _pack_lhsT(w1_full, CT, CT)
        w2_full = np.zeros((ER, ER), np.float32)
        for e in range(E):
            w2_full[e * R:(e + 1) * R, e * R:(e + 1) * R] = Cs[l, e]
        w2[l] = _pack_lhsT(w2_full, 2, 2)
        w3[l] = _pack_lhsT(Us[l].transpose(0, 2, 1).reshape(ER, C), 2, CT)
    return w1, w2, w3


_CACHED = {}


def _get_nc(with_bias):
    if with_bias not in _CACHED:
        _CACHED[with_bias] = build_kernel(with_bias)
    return _CACHED[with_bias]


def run(inputs, trace=False, trace_kwargs=None):
    x = np.asarray(inputs["x"], np.float32)
    Us = np.asarray(inputs["Us"], np.float32)
    Vs = np.asarray(inputs["Vs"], np.float32)
    Cs = np.asarray(inputs["Cs"], np.float32)
    gate_w = np.asarray(inputs["gate_w"], np.float32)
    biases = np.asarray(inputs["biases"], np.float32)
    with_bias = bool(np.any(biases))

    w1, w2, w3 = _pack_weights(Us, Vs, Cs, gate_w)
    nc = _get_nc(with_bias)

    in_maps = []
    for h in range(NCORES):
        m = {
            "xT": np.ascontiguousarray(x[h * BC:(h + 1) * BC].T),
            "w1": w1, "w2": w2, "w3": w3,
        }
        if with_bias:
            m["biasT"] = np.ascontiguousarray(biases.reshape(L, 1, C))
        in_maps.append(m)

    kw = {}
    if trace:
        kw.update(trace=True, **(trace_kwargs or {}))
    res = run_bass_kernel_spmd(nc, in_maps, core_ids=list(range(NCORES)), **kw)
    out = np.concatenate(
        [res.results[h]["outT"].T for h in range(NCORES)], axis=0)
    return np.ascontiguousarray(out, dtype=np.float32), res


def kernel(**inputs) -> np.ndarray:
    out, _ = run(inputs, trace=False)
    return out
